# revision 1
# baseline (speedup 1.0000x reference)
"""Trainium2 Bass kernel for nn_MiniDecoderBlock (B=2, T=2048, D=1024, H=16, DI=2048).

Strategy: 8-way tensor-parallel attention (2 heads/core, both batches),
one chunked ReduceScatter of the o_proj partial sums distributing tokens,
then token-sharded FFN (512 tokens/core, full d_inner).

kernel(**inputs) takes the FULL unsharded inputs and returns the FULL
output; sharding/compile/run happen inside.
"""

"""MiniDecoderBlock Trainium kernel: TP-8 attention + RS + token-sharded FFN.

Layout conventions (device side, per core):
  - Activations feature-major: xT [D, tokens] so matmul contraction (partition
    dim) is the feature dim.
  - Scores computed transposed: scoresT [k_tokens(P), q_tokens(free)] so the
    PV matmul uses stationary V and lands yT feature-major for o_proj.
  - V stored token-major with an appended ones column (sumexp for free).
  - rmsnorm applied via a PE ones-broadcast of the rms row onto all partitions,
    multiplied into q/k/v at the mandatory PSUM->SBUF copy.
  - ReduceScatter distributes attention partial sums by token blocks; core r
    owns global 128-token blocks {8c + r}.
"""

import numpy as np

import concourse.bass as bass
import concourse.mybir as mybir
import concourse.tile as tile
from concourse import bacc
from concourse.masks import make_identity
from concourse.tile import TileContext

F32 = mybir.dt.float32
F32R = mybir.dt.float32r
BF16 = mybir.dt.bfloat16

N_CORES = 8
B, T, D = 2, 2048, 1024
H, HD = 16, 64
DI = 2048
HPC = H // N_CORES          # heads per core = 2
NTOK = B * T                # 4096
NCHUNK = NTOK // 512        # 8 x 512-token chunks
NBLK = NTOK // 128          # 32 x 128-token blocks
EPS = 1e-6
NEG = -1e30


def r32(ap):
    return ap.bitcast(F32R)


def build_nc(ffn_w_dtype=BF16, reps=1, no_collective=False):
    nc = bacc.Bacc("TRN2", target_bir_lowering=False, debug=False,
                   num_devices=1 if no_collective else N_CORES)

    xT = nc.dram_tensor("xT", [D, NTOK], BF16, kind="ExternalInput")
    x_own = nc.dram_tensor("x_own", [512, D], F32, kind="ExternalInput")
    qkvT = nc.dram_tensor("qkvT", [D, 3 * HPC * HD], BF16, kind="ExternalInput")
    o_wT = nc.dram_tensor("o_wT", [HPC * HD, D], F32R, kind="ExternalInput")
    gT = nc.dram_tensor("gT", [D, DI], ffn_w_dtype, kind="ExternalInput")
    uT = nc.dram_tensor("uT", [D, DI], ffn_w_dtype, kind="ExternalInput")
    dT = nc.dram_tensor("dT", [DI, D], ffn_w_dtype, kind="ExternalInput")
    out = nc.dram_tensor("out", [512, D], F32, kind="ExternalOutput")

    with TileContext(nc) as tc:
        emit(nc, tc, xT, x_own, qkvT, o_wT, gT, uT, dT, out, reps=reps,
             no_collective=no_collective)
    nc.compile()
    return nc


def emit(nc, tc, xT, x_own, qkvT, o_wT, gT, uT, dT, out, reps=1, no_collective=False):
    EXP = mybir.ActivationFunctionType.Exp
    LN = mybir.ActivationFunctionType.Ln
    SQUARE = mybir.ActivationFunctionType.Square
    SIGMOID = mybir.ActivationFunctionType.Sigmoid
    MUL = mybir.AluOpType.mult
    ADD = mybir.AluOpType.add

    from contextlib import ExitStack
    ctx = ExitStack()
    consts = ctx.enter_context(tc.tile_pool(name="consts", bufs=1))
    dram = ctx.enter_context(tc.tile_pool(name="dram", bufs=1, space="DRAM"))
    psum = ctx.enter_context(tc.tile_pool(name="psum", bufs=2, space="PSUM"))
    sb = ctx.enter_context(tc.tile_pool(name="sb", bufs=2))

    # ---- constants ----
    ident = consts.tile([128, 128], F32, tag="ident")
    make_identity(nc, ident[:, :])
    ident_bf = consts.tile([128, 128], BF16, tag="ident_bf")
    make_identity(nc, ident_bf[:, :])
    ones_c = consts.tile([128, 1], BF16, tag="ones_c")
    nc.vector.memset(ones_c[:, :], 1.0)
    ones_r = consts.tile([1, 128], F32R, tag="ones_r")
    nc.vector.memset(ones_r[:, :].bitcast(F32), 1.0)
    eps_col = consts.tile([128, 1], F32, tag="eps_col")
    nc.vector.memset(eps_col[:, :], EPS)
    # PE-side causal masking: scores_diag = (-BIG*I).T @ ut01 + kT.T @ q
    # neg_ident = -BIG on the diagonal; ut01 = 1.0 strictly below diagonal (k>q)
    madd = consts.tile([128, 128], F32, tag="madd")
    nc.gpsimd.memset(madd[:, :], 0.0)
    nc.gpsimd.affine_select(
        out=madd[:, :], in_=madd[:, :],
        compare_op=mybir.AluOpType.is_ge, fill=NEG,
        base=0, pattern=[[1, 128]], channel_multiplier=-1,
    )

    # ---- persistent SBUF ----
    qkvT_sb = consts.tile([128, 8 * 384], BF16, tag="qkvT_sb")
    for kk in range(8):
        nc.sync.dma_start(out=qkvT_sb[:, kk * 384:(kk + 1) * 384],
                          in_=qkvT[kk * 128:(kk + 1) * 128, :])
    o_wT_sb = consts.tile([128, D], F32R, tag="o_wT_sb")
    nc.sync.dma_start(out=o_wT_sb[:, :], in_=o_wT[:, :])

    kT_all = consts.tile([128, NTOK], F32R, tag="kT_all")
    v_aug = consts.tile([128, HPC * NBLK * 65], BF16, tag="v_aug")
    nc.vector.memset(v_aug[:, :], 1.0)

    # FFN weights resident (gate/up), bf16 -- loaded during chunk 1
    g_sb = consts.tile([128, 8 * DI], gT.dtype, tag="g_sb")
    u_sb = consts.tile([128, 8 * DI], uT.dtype, tag="u_sb")

    # ---- DRAM bounce ----
    rs_in = dram.tile([NTOK, D], BF16, tag="rs_in")
    rs_out = dram.tile([512, D], BF16, tag="rs_out")

    for _rep in range(reps):
        # ================= main loop over 512-token chunks =================
        def stats(i):
            """Load xT chunk i + rms broadcast tile (emitted ~1.5 chunks ahead)."""
            csl = slice(i * 512, (i + 1) * 512)
            xt = []
            for kk in range(8):
                t = sb.tile([128, 512], BF16, tag=f"xt{kk}", name=f"xt_{i}_{kk}")
                nc.sync.dma_start(out=t[:, :],
                                  in_=xT[kk * 128:(kk + 1) * 128, csl])
                xt.append(t)
            ss = psum.tile([1, 512], F32, tag="proj", bufs=3, name=f"ss_{i}")
            for kk in range(8):
                sq = sb.tile([128, 512], BF16, tag="sq", bufs=1, name=f"sq_{i}_{kk}")
                nc.vector.tensor_tensor(out=sq[:, :], in0=xt[kk][:, :],
                                        in1=xt[kk][:, :], op=MUL)
                nc.tensor.matmul(ss[:, :], ones_c[:, :], sq[:, :],
                                 start=(kk == 0), stop=(kk == 7))
            lt = sb.tile([1, 512], F32, tag="lt", bufs=1, name=f"lt_{i}")
            nc.scalar.activation(lt[:, :], ss[:, :], LN,
                                 bias=eps_col[0:1, :], scale=1.0 / D)
            rms_row = sb.tile([1, 512], F32, tag="rms_row", name=f"rmsr_{i}")
            nc.scalar.activation(rms_row[:, :], lt[:, :], EXP, scale=-0.5)
            rms_b = sb.tile([128, 512], F32, tag="rms_b", name=f"rmsb_{i}")
            nc.gpsimd.partition_broadcast(rms_b[:, :], rms_row[0:1, :])
            return xt, rms_b

        def qkv_steps(i, st):
            """Projection for chunk i as filler closures sprinkled into the
            previous chunk's attention g-loop (PE fills exp-wait gaps)."""
            csl = slice(i * 512, (i + 1) * 512)
            xt, rms_b = st
            state = {}
            steps = []

            def mk_proj(w, off, kk):
                def f():
                    if kk == 0:
                        state[w] = psum.tile([128, 512], F32, tag="proj",
                                             bufs=3, name=f"pj_{w}_{i}")
                    nc.tensor.matmul(
                        state[w][:, :],
                        qkvT_sb[:, kk * 384 + off:kk * 384 + off + 128],
                        xt[kk][:, :],
                        start=(kk == 0), stop=(kk == 7))
                return f

            for w, off in (("q", 0), ("k", 128), ("v", 256)):
                for kk in range(8):
                    steps.append(mk_proj(w, off, kk))

            def mk_qk_epi():
                def f():
                    q_sb = sb.tile([128, 512], F32R, tag="q_sb", name=f"q_{i}")
                    state["q_sb"] = q_sb
                    nc.vector.tensor_tensor(out=q_sb[:, :], in0=state["q"][:, :],
                                            in1=rms_b[:, :], op=MUL)
                    nc.vector.tensor_tensor(out=kT_all[:, csl],
                                            in0=state["k"][:, :],
                                            in1=rms_b[:, :], op=MUL)
                    v_sb = sb.tile([128, 512], BF16, tag="v_sb", name=f"v_{i}")
                    state["v_sb"] = v_sb
                    nc.vector.tensor_tensor(out=v_sb[:, :], in0=state["v"][:, :],
                                            in1=rms_b[:, :], op=MUL)
                return f

            steps.append(mk_qk_epi())

            def mk_vt(h, j):
                def f():
                    gb = i * 4 + j
                    v_sb = state["v_sb"]
                    vt = psum.tile([128, 64], BF16, tag="proj", bufs=3)
                    nc.tensor.transpose(vt[:, :],
                                        v_sb[h * 64:(h + 1) * 64,
                                             j * 128:(j + 1) * 128],
                                        ident_bf[h * 64:(h + 1) * 64,
                                                  h * 64:(h + 1) * 64])
                    slot = (h * NBLK + gb) * 65
                    nc.vector.tensor_copy(v_aug[:, slot:slot + 64], vt[:, :])
                return f

            for h in range(HPC):
                for j in range(4):
                    steps.append(mk_vt(h, j))
            return steps, state

        def attn_both(i, q_sb, y2_sb, fillers=()):
            fillers = list(fillers)
            b, li = divmod(i, 4)
            nblk = li * 4 + 4
            per_g = max(1, -(-len(fillers) // max(1, nblk)))
            yT = [psum.tile([65, 512], F32, tag="yT", bufs=2, name=f"yT_{i}_{h}")
                  for h in range(2)]
            for g in range(nblk):
                gb = b * 16 + g
                q_off = max(0, g - li * 4) * 128
                w = 512 - q_off
                scs = []
                for h in range(2):
                    sc = psum.tile([128, 512], F32, tag="sc", bufs=3,
                                   name=f"sc{h}")
                    # lhsT at partitions h*64..h*64+64 -> distinct PE row
                    # groups; the two matmuls run concurrently in the array.
                    nc.tensor.matmul(
                        sc[:, 0:w],
                        kT_all[h * 64:(h + 1) * 64, gb * 128:(gb + 1) * 128],
                        q_sb[h * 64:(h + 1) * 64, q_off:512],
                        start=True, stop=True)
                    scs.append(sc)
                for h in range(2):
                    sc = scs[h]
                    if g >= li * 4:
                        nc.vector.tensor_tensor(out=sc[:, 0:128],
                                                in0=sc[:, 0:128],
                                                in1=madd[:, :], op=ADD)
                    pT = sb.tile([128, 512], BF16, tag="pT", bufs=3,
                                 name=f"pT{h}")
                    nc.scalar.activation(pT[:, 0:w], sc[:, 0:w], EXP)
                    slot = (h * NBLK + gb) * 65
                    nc.tensor.matmul(
                        yT[h][:, q_off:512],
                        v_aug[:, slot:slot + 65],
                        pT[:, 0:w],
                        start=(g == 0), stop=(g == nblk - 1))
                for _ in range(per_g):
                    if fillers:
                        fillers.pop(0)()
            while fillers:
                fillers.pop(0)()
            for h in range(2):
                se = sb.tile([1, 512], F32, tag="se")
                nc.vector.tensor_copy(se[:, :], yT[h][64:65, :])
                rec = sb.tile([1, 512], F32, tag="rec")
                nc.vector.reciprocal(rec[:, :], se[:, :])
                rb = sb.tile([64, 512], F32, tag="rb", bufs=1)
                nc.gpsimd.partition_broadcast(rb[:, :], rec[0:1, :])
                nc.vector.tensor_tensor(out=y2_sb[h * 64:(h + 1) * 64, :],
                                        in0=yT[h][0:64, :], in1=rb[:, :],
                                        op=MUL)

        def o_proj(i, y2_sb):
            for j in range(4):
                osb = sb.tile([128, 1024], BF16, tag="osb", bufs=2)
                for n in range(2):
                    op = psum.tile([128, 512], F32, tag="sc", bufs=3)
                    nc.tensor.matmul(op[:, :],
                                     y2_sb[:, j * 128:(j + 1) * 128],
                                     o_wT_sb[:, n * 512:(n + 1) * 512],
                                     start=True, stop=True)
                    nc.vector.tensor_copy(osb[:, n * 512:(n + 1) * 512],
                                          op[:, :])
                r0 = i * 512 + j * 128
                nc.sync.dma_start(out=rs_in[r0:r0 + 128, :], in_=osb[:, :])

        # ================= FFN on own 512 tokens =================
        def ffn_prep(ha):
            x2t, xn2T, xn2s = [], [], []
            for jj in range(2):
                c2 = ha * 2 + jj
                rsx = sb.tile([128, D], BF16, tag="rsx", bufs=1, name=f"rsx{c2}")
                nc.sync.dma_start(out=rsx[:, :],
                                  in_=rs_out[c2 * 128:(c2 + 1) * 128, :])
                xo = sb.tile([128, D], F32, tag="xo", bufs=1, name=f"xo{c2}")
                nc.sync.dma_start(out=xo[:, :],
                                  in_=x_own[c2 * 128:(c2 + 1) * 128, :])
                x2 = sb.tile([128, D], BF16, tag=f"x2_{jj}", bufs=2,
                             name=f"x2_{c2}")
                nc.vector.tensor_tensor(out=x2[:, :], in0=rsx[:, :],
                                        in1=xo[:, :], op=ADD)
                x2t.append(x2)
                scr = sb.tile([128, D], BF16, tag="scr", bufs=1, name=f"scr{c2}")
                ss2 = sb.tile([128, 1], F32, tag="ss2", name=f"ss2_{c2}")
                nc.scalar.activation(scr[:, :], x2[:, :], SQUARE,
                                     accum_out=ss2[:, :])
                t2 = sb.tile([128, 1], F32, tag="t2", name=f"t2_{c2}")
                nc.scalar.activation(t2[:, :], ss2[:, :], LN,
                                     bias=eps_col[:, :], scale=1.0 / D)
                r2 = sb.tile([128, 1], F32, tag="r2", name=f"r2_{c2}")
                nc.scalar.activation(r2[:, :], t2[:, :], EXP, scale=-0.5)
                xn2 = sb.tile([128, D], BF16, tag=f"xn2_{jj}", bufs=2,
                              name=f"xn2_{c2}")
                nc.vector.tensor_scalar_mul(xn2[:, :], x2[:, :], r2[:, :])
                xn2s.append(xn2)
            for kk in range(8):
                xt2 = sb.tile([128, 256], gT.dtype, tag=f"xn2T{kk}",
                              bufs=2, name=f"xn2T{kk}_{ha}")
                xn2T.append(xt2)

            def mk_tp(jj, kk):
                def f():
                    xp = psum.tile([128, 128], BF16, tag="proj", bufs=3)
                    nc.tensor.transpose(xp[:, :],
                                        xn2s[jj][:, kk * 128:(kk + 1) * 128],
                                        ident_bf[:, :])
                    nc.vector.tensor_copy(xn2T[kk][:, jj * 128:(jj + 1) * 128],
                                          xp[:, :])
                return f

            tps = [mk_tp(jj, kk) for jj in range(2) for kk in range(8)]
            return x2t, xn2T, tps

        def ffn_mats(ha, x2t, xn2T, fillers=()):
            fillers = list(fillers)
            h_sb = []
            for m in range(16):
                if m >= 8 and fillers:
                    fillers.pop(0)()
                    if fillers:
                        fillers.pop(0)()
                gp = psum.tile([128, 256], F32, tag="sc", bufs=3)
                up = psum.tile([128, 256], F32, tag="sc", bufs=3)
                for kk in range(8):
                    nc.tensor.matmul(gp[:, :],
                                     g_sb[:, kk * DI + m * 128:kk * DI + (m + 1) * 128],
                                     xn2T[kk][:, :],
                                     start=(kk == 0), stop=(kk == 7))
                for kk in range(8):
                    nc.tensor.matmul(up[:, :],
                                     u_sb[:, kk * DI + m * 128:kk * DI + (m + 1) * 128],
                                     xn2T[kk][:, :],
                                     start=(kk == 0), stop=(kk == 7))
                sg = sb.tile([128, 256], BF16, tag="sg")
                nc.scalar.activation(sg[:, :], gp[:, :], SIGMOID)
                nc.vector.tensor_tensor(out=sg[:, :], in0=sg[:, :],
                                        in1=gp[:, :], op=MUL)
                hm = sb.tile([128, 256], dT.dtype, tag=f"h{m}", bufs=1,
                             name=f"h{m}_{ha}")
                nc.vector.tensor_tensor(out=hm[:, :], in0=sg[:, :],
                                        in1=up[:, :], op=MUL)
                h_sb.append(hm)

            while fillers:
                fillers.pop(0)()
            dp = [psum.tile([128, 512], F32,
                            tag="proj" if nn == 0 else "sc", bufs=3,
                            name=f"dp{jj}_{nn}")
                  for nn in range(2) for jj in range(2)]
            for m in range(16):
                dt = sb.tile([128, 1024], dT.dtype, tag="dt", bufs=4)
                nc.sync.dma_start(out=dt[:, :],
                                  in_=dT[m * 128:(m + 1) * 128, :])
                for n in range(2):
                    for jj in range(2):
                        nc.tensor.matmul(dp[n * 2 + jj][:, :],
                                         h_sb[m][:, jj * 128:(jj + 1) * 128],
                                         dt[:, n * 512:(n + 1) * 512],
                                         start=(m == 0), stop=(m == 15))
            for n in range(2):
                for jj in range(2):
                    c2 = ha * 2 + jj
                    osb = sb.tile([128, 512], F32, tag="fout")
                    nc.vector.tensor_tensor(out=osb[:, :],
                                            in0=dp[n * 2 + jj][:, :],
                                            in1=x2t[jj][:, n * 512:(n + 1) * 512],
                                            op=ADD)
                    nc.sync.dma_start(out=out[c2 * 128:(c2 + 1) * 128,
                                              n * 512:(n + 1) * 512],
                                      in_=osb[:, :])


        st = stats(0)
        steps0, state0 = qkv_steps(0, st)
        for f in steps0:
            f()
        q_cur = state0["q_sb"]
        st_next = stats(1)
        state_next = None
        for i in range(NCHUNK):
            y2_sb = sb.tile([128, 512], F32R, tag="y2_sb", name=f"y2_{i}")
            if i + 1 < NCHUNK:
                fillers, state_next = qkv_steps(i + 1, st_next)
            else:
                fillers = []
            attn_both(i, q_cur, y2_sb, fillers)
            if i == 1:
                for kk in range(8):
                    nc.sync.dma_start(out=g_sb[:, kk * DI:(kk + 1) * DI],
                                      in_=gT[kk * 128:(kk + 1) * 128, :])
                    nc.sync.dma_start(out=u_sb[:, kk * DI:(kk + 1) * DI],
                                      in_=uT[kk * 128:(kk + 1) * 128, :])
            if i + 2 < NCHUNK:
                st_next = stats(i + 2)
            o_proj(i, y2_sb)
            if i + 1 < NCHUNK:
                q_cur = state_next["q_sb"]
            if i == 5:
                ffn0 = ffn_prep(0)
            if i % 2 == 1:
                c = i // 2
                if no_collective:
                    nc.sync.dma_start(
                        out=rs_out[c * 128:(c + 1) * 128, :],
                        in_=rs_in[c * 1024:c * 1024 + 128, :])
                else:
                    nc.gpsimd.collective_compute(
                        "ReduceScatter", mybir.AluOpType.add,
                        ins=[rs_in[c * 1024:(c + 1) * 1024, :]],
                        outs=[rs_out[c * 128:(c + 1) * 128, :]],
                        replica_groups=[list(range(N_CORES))],
                    )

        x2t0, xn2T0, tps0 = ffn0
        for f in tps0:
            f()
        x2t1, xn2T1, tps1 = ffn_prep(1)
        ffn_mats(0, x2t0, xn2T0, tps1)
        ffn_mats(1, x2t1, xn2T1)

    ctx.close()


# ===================== host-side sharding =====================

def make_in_maps(x, ln1_w, ln2_w, qkv_w, o_w, gate_w, up_w, down_w,
                 ffn_np_dtype=None):
    import ml_dtypes
    if ffn_np_dtype is None:
        ffn_np_dtype = ml_dtypes.bfloat16
    x = np.asarray(x, np.float32)
    xf = np.ascontiguousarray(x.reshape(NTOK, D))
    xT = np.ascontiguousarray(xf.T).astype(ml_dtypes.bfloat16)

    qkv_eff = np.asarray(qkv_w, np.float32) * np.asarray(ln1_w, np.float32)[None, :]
    g_eff = np.asarray(gate_w, np.float32) * np.asarray(ln2_w, np.float32)[None, :]
    u_eff = np.asarray(up_w, np.float32) * np.asarray(ln2_w, np.float32)[None, :]
    o_w = np.asarray(o_w, np.float32)
    down_w = np.asarray(down_w, np.float32)

    gT = np.ascontiguousarray(g_eff.T).astype(ffn_np_dtype)
    uT = np.ascontiguousarray(u_eff.T).astype(ffn_np_dtype)
    dT = np.ascontiguousarray(down_w.T).astype(ffn_np_dtype)

    scale = 1.0 / np.sqrt(HD)
    in_maps = []
    for r in range(N_CORES):
        hsl = slice(r * HPC * HD, (r + 1) * HPC * HD)  # rows for this core's heads
        qr = qkv_eff[hsl, :] * scale          # [128, D] pre-scaled q
        kr = qkv_eff[D + r * 128:D + (r + 1) * 128, :]
        vr = qkv_eff[2 * D + r * 128:2 * D + (r + 1) * 128, :]
        qkvT_r = np.ascontiguousarray(
            np.concatenate([qr, kr, vr], axis=0).T).astype(ml_dtypes.bfloat16)
        o_wT_r = np.ascontiguousarray(o_w[:, hsl].T)   # [128, D]
        xo = np.ascontiguousarray(
            xf.reshape(NBLK, 128, D)[r::N_CORES].reshape(512, D))
        in_maps.append({
            "xT": xT, "x_own": xo, "qkvT": qkvT_r, "o_wT": o_wT_r,
            "gT": gT, "uT": uT, "dT": dT,
        })
    return in_maps


def assemble_out(results):
    outf = np.empty((NTOK, D), np.float32)
    for r in range(N_CORES):
        outf.reshape(NBLK, 128, D)[r::N_CORES] = \
            results[r]["out"].reshape(4, 128, D)
    return outf.reshape(B, T, D)


# ===================== entry point =====================

_NC_CACHE = {}


def _get_nc():
    if "nc" not in _NC_CACHE:
        _NC_CACHE["nc"] = build_nc()
    return _NC_CACHE["nc"]


def kernel(x, ln1_w, ln2_w, qkv_w, o_w, gate_w, up_w, down_w):
    from concourse.bass_utils import run_bass_kernel_spmd

    nc = _get_nc()
    in_maps = make_in_maps(x, ln1_w, ln2_w, qkv_w, o_w, gate_w, up_w, down_w)
    res = run_bass_kernel_spmd(nc, in_maps, core_ids=list(range(N_CORES)))
    return assemble_out(res.results)



# revision 18
# speedup vs baseline: 1.3962x; 1.3962x over previous
"""Trainium2 Bass kernel for nn_MiniDecoderBlock (B=2, T=2048, D=1024, H=16, DI=2048).

Strategy: 8-way tensor-parallel attention (2 heads/core, both batches),
one chunked ReduceScatter of the o_proj partial sums distributing tokens,
then token-sharded FFN (512 tokens/core, full d_inner).

kernel(**inputs) takes the FULL unsharded inputs and returns the FULL
output; sharding/compile/run happen inside.
"""

"""MiniDecoderBlock Trainium kernel: TP-8 attention + RS + token-sharded FFN.

Layout conventions (device side, per core):
  - Activations feature-major: xT [D, tokens] so matmul contraction (partition
    dim) is the feature dim.
  - Scores computed transposed: scoresT [k_tokens(P), q_tokens(free)] so the
    PV matmul uses stationary V and lands yT feature-major for o_proj.
  - V stored token-major with an appended ones column (sumexp for free).
  - rmsnorm applied via a PE ones-broadcast of the rms row onto all partitions,
    multiplied into q/k/v at the mandatory PSUM->SBUF copy.
  - ReduceScatter distributes attention partial sums by token blocks; core r
    owns global 128-token blocks {8c + r}.
"""

import numpy as np

import concourse.bass as bass
import concourse.mybir as mybir
import concourse.tile as tile
from concourse import bacc
from concourse.masks import make_identity
from concourse.tile import TileContext

F32 = mybir.dt.float32
F32R = mybir.dt.float32r
BF16 = mybir.dt.bfloat16

N_CORES = 8
B, T, D = 2, 2048, 1024
H, HD = 16, 64
DI = 2048
HPC = H // N_CORES          # heads per core = 2
NTOK = B * T                # 4096
NCHUNK = NTOK // 512        # 8 x 512-token chunks
NBLK = NTOK // 128          # 32 x 128-token blocks
EPS = 1e-6
NEG = -1e30


def r32(ap):
    return ap.bitcast(F32R)


_TABLES_PATCHED = [False]


def _patch_act_tables():
    """Make exp/ln resolvable ONLY via the combined natural_log_exp set so
    the table-load fixpoint keeps one table across the whole attention phase
    (one switch to the silu set at FFN).  Order (and thus positional
    act_func_set_id) is preserved; only set membership is filtered."""
    if _TABLES_PATCHED[0]:
        return
    import concourse.bacc as bacc_mod
    import concourse.mybir as mb
    orig = bacc_mod.get_activation_tables
    keep = "natural_log_exp_and_others"
    strip = {mb.ActivationFunctionType.Exp, mb.ActivationFunctionType.Ln}

    def patched(arch):
        t = orig(arch)
        out = {}
        for k, fns in t.items():
            out[k] = set(fns) if k == keep else set(fns) - strip
        return out

    bacc_mod.get_activation_tables = patched
    _TABLES_PATCHED[0] = True


def build_nc(ffn_w_dtype=BF16, reps=1, no_collective=False):
    _patch_act_tables()
    nc = bacc.Bacc("TRN2", target_bir_lowering=False, debug=False,
                   num_devices=1 if no_collective else N_CORES)

    xT = nc.dram_tensor("xT", [D, NTOK], BF16, kind="ExternalInput")
    x_own = nc.dram_tensor("x_own", [512, D], BF16, kind="ExternalInput")
    qkvT = nc.dram_tensor("qkvT", [D, 3 * HPC * HD], BF16, kind="ExternalInput")
    o_wT = nc.dram_tensor("o_wT", [HPC * HD, D], BF16, kind="ExternalInput")
    gT = nc.dram_tensor("gT", [D, DI], ffn_w_dtype, kind="ExternalInput")
    uT = nc.dram_tensor("uT", [D, DI], ffn_w_dtype, kind="ExternalInput")
    dT = nc.dram_tensor("dT", [DI, D], ffn_w_dtype, kind="ExternalInput")
    out = nc.dram_tensor("out", [512, D], F32, kind="ExternalOutput")

    with TileContext(nc) as tc:
        emit(nc, tc, xT, x_own, qkvT, o_wT, gT, uT, dT, out, reps=reps,
             no_collective=no_collective)
    nc.compile()
    return nc


def emit(nc, tc, xT, x_own, qkvT, o_wT, gT, uT, dT, out, reps=1, no_collective=False):
    EXP = mybir.ActivationFunctionType.Exp
    LN = mybir.ActivationFunctionType.Ln
    SQUARE = mybir.ActivationFunctionType.Square
    SILU = mybir.ActivationFunctionType.Silu
    MUL = mybir.AluOpType.mult
    ADD = mybir.AluOpType.add

    from contextlib import ExitStack
    ctx = ExitStack()
    consts = ctx.enter_context(tc.tile_pool(name="consts", bufs=1))
    dram = ctx.enter_context(tc.tile_pool(name="dram", bufs=1, space="DRAM"))
    psum = ctx.enter_context(tc.tile_pool(name="psum", bufs=2, space="PSUM"))
    sb = ctx.enter_context(tc.tile_pool(name="sb", bufs=2))

    # ---- constants ----
    ident_bf = consts.tile([128, 128], BF16, tag="ident_bf")
    make_identity(nc, ident_bf[:, :])
    ones_c = consts.tile([128, 1], BF16, tag="ones_c")
    nc.vector.memset(ones_c[:, :], 1.0)
    eps_col = consts.tile([128, 1], F32, tag="eps_col")
    nc.vector.memset(eps_col[:, :], EPS)
    # PE-side causal masking: scores_diag = (-BIG*I).T @ ut01 + kT.T @ q
    # neg_ident = -BIG on the diagonal; ut01 = 1.0 strictly below diagonal (k>q)
    madd = consts.tile([128, 128], F32, tag="madd")
    nc.gpsimd.memset(madd[:, :], 0.0)
    nc.gpsimd.affine_select(
        out=madd[:, :], in_=madd[:, :],
        compare_op=mybir.AluOpType.is_ge, fill=NEG,
        base=0, pattern=[[1, 128]], channel_multiplier=-1,
    )

    # ---- persistent SBUF ----
    qkvT_sb = consts.tile([128, 8 * 384], BF16, tag="qkvT_sb")
    for kk in range(8):
        nc.sync.dma_start(out=qkvT_sb[:, kk * 384:(kk + 1) * 384],
                          in_=qkvT[kk * 128:(kk + 1) * 128, :])
    o_wT_sb = consts.tile([128, D], BF16, tag="o_wT_sb")
    nc.sync.dma_start(out=o_wT_sb[:, :], in_=o_wT[:, :])

    kT_all = consts.tile([128, NTOK // 2], BF16, tag="kT_all")
    v_aug = consts.tile([128, HPC * NBLK * 65], BF16, tag="v_aug")
    nc.vector.memset(v_aug[:, :], 1.0)

    # FFN weights resident (gate/up/down), bf16 -- loaded once at start
    g_sb = consts.tile([128, 8 * DI], gT.dtype, tag="g_sb")
    u_sb = consts.tile([128, 8 * DI], uT.dtype, tag="u_sb")
    dt_all = consts.tile([128, 16 * D], dT.dtype, tag="dt_all")

    def load_weights_piece(i):
        # first-rep staged weight residency: g/u over chunks 0-3, dT 4-7
        nc.sync.dma_start(out=g_sb[:, i * DI:(i + 1) * DI],
                          in_=gT[i * 128:(i + 1) * 128, :])
        nc.gpsimd.dma_start(out=u_sb[:, i * DI:(i + 1) * DI],
                            in_=uT[i * 128:(i + 1) * 128, :])
        if i >= 4:
            q = i - 4
            nc.gpsimd.dma_start(
                out=dt_all[:, q * 4 * D:(q + 1) * 4 * D].rearrange(
                    "p (m d) -> p m d", m=4),
                in_=dT[q * 512:(q + 1) * 512, :].rearrange(
                    "(m p) d -> p m d", p=128))

    # ---- DRAM bounce ----
    rs_in = dram.tile([NTOK, D], BF16, tag="rs_in")
    rs_out = dram.tile([512, D], BF16, tag="rs_out")

    for _rep in range(reps):
        # ================= main loop over 512-token chunks =================
        def stats(i):
            """Load xT chunk i + rms broadcast tile (emitted ~1.5 chunks ahead)."""
            csl = slice(i * 512, (i + 1) * 512)
            xt = []
            for kk in range(8):
                t = sb.tile([128, 512], BF16, tag=f"xt{kk}", name=f"xt_{i}_{kk}")
                nc.sync.dma_start(out=t[:, :],
                                  in_=xT[kk * 128:(kk + 1) * 128, csl])
                xt.append(t)
            ss = psum.tile([1, 512], F32, tag="proj", bufs=3, name=f"ss_{i}")
            for kk in range(8):
                sq = sb.tile([128, 512], BF16, tag="sq", bufs=1, name=f"sq_{i}_{kk}")
                nc.vector.tensor_tensor(out=sq[:, :], in0=xt[kk][:, :],
                                        in1=xt[kk][:, :], op=MUL)
                nc.tensor.matmul(ss[:, :], ones_c[:, :], sq[:, :],
                                 start=(kk == 0), stop=(kk == 7))
            lt = sb.tile([1, 512], F32, tag="lt", bufs=1, name=f"lt_{i}")
            nc.scalar.activation(lt[:, :], ss[:, :], LN,
                                 bias=eps_col[0:1, :], scale=1.0 / D)
            rms_row = sb.tile([1, 512], BF16, tag="rms_row", name=f"rmsr_{i}")
            nc.scalar.activation(rms_row[:, :], lt[:, :], EXP, scale=-0.5)
            rms_b = sb.tile([128, 512], BF16, tag="rms_b", name=f"rmsb_{i}")
            nc.gpsimd.partition_broadcast(rms_b[:, :], rms_row[0:1, :])
            return xt, rms_b

        def qkv_steps(i, st):
            """Projection for chunk i as filler closures sprinkled into the
            previous chunk's attention g-loop (PE fills exp-wait gaps)."""
            csl = slice(i * 512, (i + 1) * 512)
            xt, rms_b = st
            state = {}
            steps = []

            def mk_proj(w, off, kk):
                def f():
                    if kk == 0:
                        state[w] = psum.tile([128, 512], F32, tag="proj",
                                             bufs=3, name=f"pj_{w}_{i}")
                    nc.tensor.matmul(
                        state[w][:, :],
                        qkvT_sb[:, kk * 384 + off:kk * 384 + off + 128],
                        xt[kk][:, :],
                        start=(kk == 0), stop=(kk == 7))
                return f

            for w, off in (("q", 0), ("k", 128), ("v", 256)):
                for kk in range(8):
                    steps.append(mk_proj(w, off, kk))

            def mk_qk_epi():
                def f():
                    q_sb = sb.tile([128, 512], BF16, tag="q_sb", name=f"q_{i}")
                    state["q_sb"] = q_sb
                    nc.vector.tensor_tensor(out=q_sb[:, :], in0=state["q"][:, :],
                                            in1=rms_b[:, :], op=MUL)
                    ksl = slice((i % 4) * 512, (i % 4) * 512 + 512)
                    nc.vector.tensor_tensor(out=kT_all[:, ksl],
                                            in0=state["k"][:, :],
                                            in1=rms_b[:, :], op=MUL)
                    v_sb = sb.tile([128, 512], BF16, tag="v_sb", name=f"v_{i}")
                    state["v_sb"] = v_sb
                    nc.vector.tensor_tensor(out=v_sb[:, :], in0=state["v"][:, :],
                                            in1=rms_b[:, :], op=MUL)
                return f

            steps.append(mk_qk_epi())

            def mk_vt(h, j):
                def f():
                    gb = i * 4 + j
                    v_sb = state["v_sb"]
                    vt = psum.tile([128, 64], BF16, tag="proj", bufs=3)
                    nc.tensor.transpose(vt[:, :],
                                        v_sb[h * 64:(h + 1) * 64,
                                             j * 128:(j + 1) * 128],
                                        ident_bf[h * 64:(h + 1) * 64,
                                                  h * 64:(h + 1) * 64])
                    slot = (h * NBLK + gb) * 65
                    nc.vector.tensor_copy(v_aug[:, slot:slot + 64], vt[:, :])
                return f

            for h in range(HPC):
                for j in range(4):
                    steps.append(mk_vt(h, j))
            return steps, state

        def attn_both(i, q_sb, y2_sb, fillers=()):
            fillers = list(fillers)
            b, li = divmod(i, 4)
            nblk = li * 4 + 4
            per_g = max(1, -(-len(fillers) // max(1, nblk)))
            yT = [psum.tile([65, 512], F32, tag="yT", bufs=2, name=f"yT_{i}_{h}")
                  for h in range(2)]
            for g in range(nblk):
                gb = b * 16 + g
                q_off = max(0, g - li * 4) * 128
                w = 512 - q_off
                scs = []
                for h in range(2):
                    sc = psum.tile([128, 512], F32, tag="sc", bufs=3,
                                   name=f"sc{h}")
                    # lhsT at partitions h*64..h*64+64 -> distinct PE row
                    # groups; the two matmuls run concurrently in the array.
                    nc.tensor.matmul(
                        sc[:, 0:w],
                        kT_all[h * 64:(h + 1) * 64,
                               (gb % 16) * 128:(gb % 16) * 128 + 128],
                        q_sb[h * 64:(h + 1) * 64, q_off:512],
                        start=True, stop=True)
                    scs.append(sc)
                for h in range(2):
                    sc = scs[h]
                    if g >= li * 4:
                        nc.vector.tensor_tensor(out=sc[:, 0:128],
                                                in0=sc[:, 0:128],
                                                in1=madd[:, :], op=ADD)
                    pT = sb.tile([128, 512], BF16, tag="pT", bufs=3,
                                 name=f"pT{h}")
                    nc.scalar.activation(pT[:, 0:w], sc[:, 0:w], EXP)
                    slot = (h * NBLK + gb) * 65
                    nc.tensor.matmul(
                        yT[h][:, q_off:512],
                        v_aug[:, slot:slot + 65],
                        pT[:, 0:w],
                        start=(g == 0), stop=(g == nblk - 1))
                for _ in range(per_g):
                    if fillers:
                        fillers.pop(0)()
            while fillers:
                fillers.pop(0)()
            for h in range(2):
                se = sb.tile([1, 512], F32, tag="se")
                nc.vector.tensor_copy(se[:, :], yT[h][64:65, :])
                rec = sb.tile([1, 512], BF16, tag="rec")
                with nc.allow_low_precision(reason="bf16 1/sumexp"):
                    nc.vector.reciprocal(rec[:, :], se[:, :])
                rb = sb.tile([64, 512], BF16, tag="rb", bufs=1)
                nc.gpsimd.partition_broadcast(rb[:, :], rec[0:1, :])
                nc.vector.tensor_tensor(out=y2_sb[h * 64:(h + 1) * 64, :],
                                        in0=yT[h][0:64, :], in1=rb[:, :],
                                        op=MUL)

        def o_proj(i, y2_sb):
            for j in range(4):
                osb = sb.tile([128, 1024], BF16, tag="osb", bufs=2)
                for n in range(2):
                    op = psum.tile([128, 512], F32, tag="sc", bufs=3)
                    nc.tensor.matmul(op[:, :],
                                     y2_sb[:, j * 128:(j + 1) * 128],
                                     o_wT_sb[:, n * 512:(n + 1) * 512],
                                     start=True, stop=True)
                    nc.vector.tensor_copy(osb[:, n * 512:(n + 1) * 512],
                                          op[:, :])
                r0 = i * 512 + j * 128
                nc.sync.dma_start(out=rs_in[r0:r0 + 128, :], in_=osb[:, :])

        # ================= FFN on own 512 tokens =================
        def ffn_prep(ha):
            x2t, xn2T, xn2s = [], [], []
            for jj in range(2):
                c2 = ha * 2 + jj
                rsx = sb.tile([128, D], BF16, tag="rsx", bufs=1, name=f"rsx{c2}")
                nc.sync.dma_start(out=rsx[:, :],
                                  in_=rs_out[c2 * 128:(c2 + 1) * 128, :])
                xo = sb.tile([128, D], BF16, tag="xo", bufs=1, name=f"xo{c2}")
                nc.sync.dma_start(out=xo[:, :],
                                  in_=x_own[c2 * 128:(c2 + 1) * 128, :])
                x2 = sb.tile([128, D], BF16, tag=f"x2_{jj}", bufs=2,
                             name=f"x2_{c2}")
                nc.vector.tensor_tensor(out=x2[:, :], in0=rsx[:, :],
                                        in1=xo[:, :], op=ADD)
                x2t.append(x2)
                scr = sb.tile([128, D], BF16, tag="scr", bufs=1, name=f"scr{c2}")
                ss2 = sb.tile([128, 1], F32, tag="ss2", name=f"ss2_{c2}")
                nc.scalar.activation(scr[:, :], x2[:, :], SQUARE,
                                     accum_out=ss2[:, :])
                t2 = sb.tile([128, 1], F32, tag="t2", name=f"t2_{c2}")
                nc.scalar.activation(t2[:, :], ss2[:, :], LN,
                                     bias=eps_col[:, :], scale=1.0 / D)
                r2 = sb.tile([128, 1], F32, tag="r2", name=f"r2_{c2}")
                nc.scalar.activation(r2[:, :], t2[:, :], EXP, scale=-0.5)
                xn2 = sb.tile([128, D], BF16, tag=f"xn2_{jj}", bufs=2,
                              name=f"xn2_{c2}")
                nc.vector.tensor_scalar_mul(xn2[:, :], x2[:, :], r2[:, :])
                xn2s.append(xn2)
            for kk in range(8):
                xt2 = sb.tile([128, 256], gT.dtype, tag=f"xn2T{kk}",
                              bufs=2, name=f"xn2T{kk}_{ha}")
                xn2T.append(xt2)

            def mk_tp(jj, kk):
                def f():
                    xp = psum.tile([128, 128], BF16, tag="proj", bufs=3)
                    nc.tensor.transpose(xp[:, :],
                                        xn2s[jj][:, kk * 128:(kk + 1) * 128],
                                        ident_bf[:, :])
                    nc.vector.tensor_copy(xn2T[kk][:, jj * 128:(jj + 1) * 128],
                                          xp[:, :])
                return f

            tps = [mk_tp(jj, kk) for jj in range(2) for kk in range(8)]
            return x2t, xn2T, tps

        def ffn_mats(ha, x2t, xn2T, fillers=()):
            fillers = list(fillers)
            h_sb = []
            for m in range(16):
                if m >= 8 and fillers:
                    fillers.pop(0)()
                    if fillers:
                        fillers.pop(0)()
                gp = psum.tile([128, 256], F32, tag="sc", bufs=3)
                up = psum.tile([128, 256], F32, tag="sc", bufs=3)
                for kk in range(8):
                    nc.tensor.matmul(gp[:, :],
                                     g_sb[:, kk * DI + m * 128:kk * DI + (m + 1) * 128],
                                     xn2T[kk][:, :],
                                     start=(kk == 0), stop=(kk == 7))
                for kk in range(8):
                    nc.tensor.matmul(up[:, :],
                                     u_sb[:, kk * DI + m * 128:kk * DI + (m + 1) * 128],
                                     xn2T[kk][:, :],
                                     start=(kk == 0), stop=(kk == 7))
                sg = sb.tile([128, 256], BF16, tag="sg")
                nc.scalar.activation(sg[:, :], gp[:, :], SILU)
                hm = sb.tile([128, 256], dT.dtype, tag=f"h{m}", bufs=1,
                             name=f"h{m}_{ha}")
                nc.vector.tensor_tensor(out=hm[:, :], in0=sg[:, :],
                                        in1=up[:, :], op=MUL)
                h_sb.append(hm)

            while fillers:
                fillers.pop(0)()
            dp = [psum.tile([128, 512], F32,
                            tag="proj" if nn == 0 else "sc", bufs=3,
                            name=f"dp{jj}_{nn}")
                  for nn in range(2) for jj in range(2)]
            for m in range(16):
                for n in range(2):
                    for jj in range(2):
                        nc.tensor.matmul(dp[n * 2 + jj][:, :],
                                         h_sb[m][:, jj * 128:(jj + 1) * 128],
                                         dt_all[:, m * D + n * 512:m * D + (n + 1) * 512],
                                         start=(m == 0), stop=(m == 15))
            for n in range(2):
                for jj in range(2):
                    c2 = ha * 2 + jj
                    osb = sb.tile([128, 512], F32, tag="fout")
                    nc.vector.tensor_tensor(out=osb[:, :],
                                            in0=dp[n * 2 + jj][:, :],
                                            in1=x2t[jj][:, n * 512:(n + 1) * 512],
                                            op=ADD)
                    nc.sync.dma_start(out=out[c2 * 128:(c2 + 1) * 128,
                                              n * 512:(n + 1) * 512],
                                      in_=osb[:, :])


        st = stats(0)
        steps0, state0 = qkv_steps(0, st)
        for f in steps0:
            f()
        q_cur = state0["q_sb"]
        st_next = stats(1)
        state_next = None
        for i in range(NCHUNK):
            y2_sb = sb.tile([128, 512], BF16, tag="y2_sb", name=f"y2_{i}")
            if i + 1 < NCHUNK:
                fillers, state_next = qkv_steps(i + 1, st_next)
            else:
                fillers = []
            attn_both(i, q_cur, y2_sb, fillers)
            if _rep == 0:
                load_weights_piece(i)
            if i + 2 < NCHUNK:
                st_next = stats(i + 2)
            o_proj(i, y2_sb)
            if i + 1 < NCHUNK:
                q_cur = state_next["q_sb"]
            if i == 5:
                ffn0 = ffn_prep(0)
            if i % 2 == 1:
                c = i // 2
                if no_collective:
                    nc.sync.dma_start(
                        out=rs_out[c * 128:(c + 1) * 128, :],
                        in_=rs_in[c * 1024:c * 1024 + 128, :])
                else:
                    nc.gpsimd.collective_compute(
                        "ReduceScatter", mybir.AluOpType.add,
                        ins=[rs_in[c * 1024:(c + 1) * 1024, :]],
                        outs=[rs_out[c * 128:(c + 1) * 128, :]],
                        replica_groups=[list(range(N_CORES))],
                    )

        x2t0, xn2T0, tps0 = ffn0
        for f in tps0:
            f()
        x2t1, xn2T1, tps1 = ffn_prep(1)
        ffn_mats(0, x2t0, xn2T0, tps1)
        ffn_mats(1, x2t1, xn2T1)

    ctx.close()


# ===================== host-side sharding =====================

def make_in_maps(x, ln1_w, ln2_w, qkv_w, o_w, gate_w, up_w, down_w,
                 ffn_np_dtype=None):
    import ml_dtypes
    if ffn_np_dtype is None:
        ffn_np_dtype = ml_dtypes.bfloat16
    x = np.asarray(x, np.float32)
    xf = np.ascontiguousarray(x.reshape(NTOK, D))
    xT = np.ascontiguousarray(xf.T).astype(ml_dtypes.bfloat16)

    qkv_eff = np.asarray(qkv_w, np.float32) * np.asarray(ln1_w, np.float32)[None, :]
    g_eff = np.asarray(gate_w, np.float32) * np.asarray(ln2_w, np.float32)[None, :]
    u_eff = np.asarray(up_w, np.float32) * np.asarray(ln2_w, np.float32)[None, :]
    o_w = np.asarray(o_w, np.float32)
    down_w = np.asarray(down_w, np.float32)

    gT = np.ascontiguousarray(g_eff.T).astype(ffn_np_dtype)
    uT = np.ascontiguousarray(u_eff.T).astype(ffn_np_dtype)
    dT = np.ascontiguousarray(down_w.T).astype(ffn_np_dtype)

    scale = 1.0 / np.sqrt(HD)
    in_maps = []
    for r in range(N_CORES):
        hsl = slice(r * HPC * HD, (r + 1) * HPC * HD)  # rows for this core's heads
        qr = qkv_eff[hsl, :] * scale          # [128, D] pre-scaled q
        kr = qkv_eff[D + r * 128:D + (r + 1) * 128, :]
        vr = qkv_eff[2 * D + r * 128:2 * D + (r + 1) * 128, :]
        qkvT_r = np.ascontiguousarray(
            np.concatenate([qr, kr, vr], axis=0).T).astype(ml_dtypes.bfloat16)
        o_wT_r = np.ascontiguousarray(o_w[:, hsl].T).astype(ml_dtypes.bfloat16)
        xo = np.ascontiguousarray(
            xf.reshape(NBLK, 128, D)[r::N_CORES].reshape(512, D)).astype(
                ml_dtypes.bfloat16)
        in_maps.append({
            "xT": xT, "x_own": xo, "qkvT": qkvT_r, "o_wT": o_wT_r,
            "gT": gT, "uT": uT, "dT": dT,
        })
    return in_maps


def assemble_out(results):
    outf = np.empty((NTOK, D), np.float32)
    for r in range(N_CORES):
        outf.reshape(NBLK, 128, D)[r::N_CORES] = \
            results[r]["out"].reshape(4, 128, D)
    return outf.reshape(B, T, D)


# ===================== entry point =====================

_NC_CACHE = {}


def _get_nc():
    if "nc" not in _NC_CACHE:
        _NC_CACHE["nc"] = build_nc()
    return _NC_CACHE["nc"]


def kernel(x, ln1_w, ln2_w, qkv_w, o_w, gate_w, up_w, down_w):
    from concourse.bass_utils import run_bass_kernel_spmd

    nc = _get_nc()
    in_maps = make_in_maps(x, ln1_w, ln2_w, qkv_w, o_w, gate_w, up_w, down_w)
    res = run_bass_kernel_spmd(nc, in_maps, core_ids=list(range(N_CORES)))
    return assemble_out(res.results)

